# revision 1
# baseline (speedup 1.0000x reference)
"""2-layer GCN (GCNConv -> relu -> GCNConv -> mean) on 8 trn2 NeuronCores.

Math restructure:
  reference output = mean_n(h2[n]) with h2 = A_norm @ (h1 @ W2) + b2,
  h1 = relu(A_norm @ (x @ W1) + b1), A_norm = D^-1/2 (A+I) D^-1/2.
  Since mean is linear and every edge lands on exactly one dst:
    mean(h2) = (1/N) * (sum_n w_n * h1[n]) @ W2 + b2,
    w_n = dis_n * sum_{e: src_e = n} dis_{dst_e}   (edges incl. self-loops)
  so only layer 1 needs real message passing.

Device work per core (SPMD, same program, different data):
  phase 1: h' = (dis * x) @ W1 for ALL nodes (replicated; avoids
           collectives), stored to local DRAM as fp16 rows. Matmul column
           selection is Q-way interleaved so each SBUF partition ends up
           holding Q consecutive hp rows -> contiguous multi-KB store
           descriptors instead of 256B ones.
  phase 2: dst nodes are degree-sorted and assigned to (core, slot,
           partition); tile rank r -> core r%8, slot r//8, so every slot's
           per-node edge capacity C_t (max degree in that slot's 8 tiles)
           is small and the padding overhead stays low. One indirect-DMA
           gather per slot lands each dst node's edge-source rows in its
           own partition; segment-sum is then an in-place pairwise tree of
           DVE adds (no one-hot matmuls at all). Epilogue: *dis_dst, +b1,
           relu, then a [128,1]^T @ [128,128] matmul accumulates the
           w-weighted node sum into a persistent [1,128] PSUM accumulator.
  host:    sum the 8 partial vectors, /N, @W2, +b2.
"""

import sys

sys.path.insert(0, "/opt/trn_rl_repo")

from contextlib import ExitStack

import numpy as np

import concourse.tile as tile
from concourse import bacc, bass, mybir
from concourse.bass import IndirectOffsetOnAxis
from concourse.bass_utils import run_bass_kernel_spmd

N = 50000
P = 128
NCORES = 8
TILES = 392            # ceil(N / P) rounded up to a multiple of NCORES
NPAD = TILES * P       # 50176
TPC = TILES // NCORES  # 49 dst tiles (slots) per core
NPC = TPC * P          # 6272 nodes per core
D = 128                # feature dim (in & hidden)

F16 = mybir.dt.float16
F32 = mybir.dt.float32
I32 = mybir.dt.int32

_compiled = {}


def _build(c_slots):
    """Build the Bass program. c_slots[t] = edge K-capacity of slot t."""
    sumc = int(sum(c_slots))
    cmax = int(max(c_slots))
    nc = bacc.Bacc(
        "TRN2", target_bir_lowering=False, debug=False, num_devices=NCORES
    )
    xt = nc.declare_dram_parameter("xt", [P, NPAD], F16, isOutput=False)
    w1 = nc.declare_dram_parameter("w1", [P, D], F16, isOutput=False)
    idx = nc.declare_dram_parameter("idx", [P, sumc], I32, isOutput=False)
    disv = nc.declare_dram_parameter("disv", [P, TPC], F32, isOutput=False)
    wv = nc.declare_dram_parameter("wv", [P, TPC], F16, isOutput=False)
    b1b = nc.declare_dram_parameter("b1b", [P, D], F32, isOutput=False)
    sout = nc.declare_dram_parameter("sout", [1, D], F32, isOutput=True)

    hp = nc.dram_tensor("hp", [NPAD, D], F16)  # internal: scaled hidden acts

    with tile.TileContext(nc) as tc, ExitStack() as ctx:
        const = ctx.enter_context(tc.tile_pool(name="const", bufs=1))
        xpool = ctx.enter_context(tc.tile_pool(name="xchunk", bufs=3))
        p1ps = ctx.enter_context(tc.tile_pool(name="p1ps", bufs=4, space="PSUM"))
        hpool = ctx.enter_context(tc.tile_pool(name="hstore", bufs=3))
        gpool = ctx.enter_context(tc.tile_pool(name="gather", bufs=3))
        sps = ctx.enter_context(tc.tile_pool(name="sps", bufs=1, space="PSUM"))
        epool = ctx.enter_context(tc.tile_pool(name="epi", bufs=3))
        opool = ctx.enter_context(tc.tile_pool(name="outp", bufs=1))

        # ---- resident constants ----
        w1_sb = const.tile([P, D], F16)
        nc.sync.dma_start(out=w1_sb[:], in_=w1[:])
        b1b_sb = const.tile([P, D], F32)
        nc.sync.dma_start(out=b1b_sb[:], in_=b1b[:])
        disv_sb = const.tile([P, TPC], F32)
        nc.sync.dma_start(out=disv_sb[:], in_=disv[:])
        wv_sb = const.tile([P, TPC], F16)
        nc.sync.dma_start(out=wv_sb[:], in_=wv[:])
        idx_sb = const.tile([P, sumc], I32)
        nc.sync.dma_start(out=idx_sb[:], in_=idx[:])

        # ---- phase 1: h' = (dis*x) @ W1 over all NPAD nodes ----
        CH = 2048  # nodes per chunk == store group
        n0 = 0
        while n0 < NPAD:
            ch = min(CH, NPAD - n0)
            q_iv = ch // P  # row interleave factor (16 or 8)
            xc = xpool.tile([P, CH], F16, tag="xchunk")
            nc.sync.dma_start(out=xc[:, :ch], in_=xt[:, n0 : n0 + ch])
            # 3D view: xv[k, j, p] = x feature k of node n0 + p*q_iv + j
            xv = xc[:, :ch].rearrange("k (p q) -> k q p", q=q_iv)
            hs = hpool.tile([P, CH], F16, tag="hstore")
            for g in range(ch // 512):
                ps = p1ps.tile([P, 512], F32)
                for q in range(4):
                    j = g * 4 + q
                    nc.tensor.matmul(
                        out=ps[:, q * 128 : (q + 1) * 128],
                        lhsT=xv[:, j, :],
                        rhs=w1_sb[:],
                        start=True,
                        stop=True,
                    )
                nc.scalar.activation(
                    out=hs[:, g * 512 : (g + 1) * 512],
                    in_=ps[:],
                    func=mybir.ActivationFunctionType.Copy,
                )
            # partition p holds nodes n0 + p*q_iv .. n0 + p*q_iv + q_iv-1
            # in order -> per-partition contiguous q_iv*256B descriptor.
            nc.sync.dma_start(
                out=hp[n0 : n0 + ch, :].rearrange("(p q) f -> p (q f)", q=q_iv),
                in_=hs[:, :ch],
            )
            n0 += ch

        # ---- phase 2: gather + per-partition segment sum per slot ----
        s_ps = sps.tile([1, D], F32)
        off = 0
        for t in range(TPC):
            c = int(c_slots[t])
            gath = gpool.tile([P, cmax * D], F16, tag="gather")
            nc.gpsimd.indirect_dma_start(
                out=gath[:, : c * D],
                out_offset=None,
                in_=hp[:],
                in_offset=IndirectOffsetOnAxis(
                    ap=idx_sb[:, off : off + c], axis=0
                ),
            )
            # in-place pairwise tree: fold top half onto bottom half
            cur = c
            while cur > 1:
                h = cur // 2
                r = cur - h
                nc.vector.tensor_tensor(
                    out=gath[:, : h * D],
                    in0=gath[:, : h * D],
                    in1=gath[:, r * D : cur * D],
                    op=mybir.AluOpType.add,
                )
                cur = r
            # epilogue: o1 = relu(dis_dst * agg + b1)  (fp16 out)
            t1 = epool.tile([P, D], F32, tag="t1")
            nc.vector.tensor_scalar_mul(
                t1[:], gath[:, :D], disv_sb[:, t : t + 1]
            )
            nc.vector.tensor_tensor(
                out=t1[:], in0=t1[:], in1=b1b_sb[:], op=mybir.AluOpType.add
            )
            o1 = epool.tile([P, D], F16, tag="o1")
            nc.scalar.activation(
                out=o1[:], in_=t1[:], func=mybir.ActivationFunctionType.Relu
            )
            # collapsed layer 2: s += w_slot^T @ o1
            nc.tensor.matmul(
                out=s_ps[:],
                lhsT=wv_sb[:, t : t + 1],
                rhs=o1[:],
                start=(t == 0),
                stop=(t == TPC - 1),
                skip_group_check=True,
            )
            off += c

        s_sb = opool.tile([1, D], F32)
        nc.vector.tensor_copy(out=s_sb[:], in_=s_ps[:])
        nc.sync.dma_start(out=sout[:], in_=s_sb[:])

    nc.compile()
    return nc


def _prep(x, edge_index):
    """Host-side graph preprocessing -> per-core device input maps."""
    src = np.asarray(edge_index[0], dtype=np.int64)
    dst = np.asarray(edge_index[1], dtype=np.int64)
    loop = np.arange(N, dtype=np.int64)
    src_all = np.concatenate([src, loop])
    dst_all = np.concatenate([dst, loop])

    deg = np.bincount(dst_all, minlength=NPAD).astype(np.int64)
    dis = np.zeros(NPAD, dtype=np.float64)
    nz = deg > 0
    dis[nz] = 1.0 / np.sqrt(deg[nz])

    acc = np.zeros(NPAD, dtype=np.float64)
    np.add.at(acc, src_all, dis[dst_all])
    w = dis * acc  # layer-2 collapsed per-node weight

    # degree-sorted relabeling: rank r -> node perm[r];
    # tile rank rt = r // P -> core rt % 8, slot rt // 8, partition r % P.
    perm = np.argsort(-deg, kind="stable")
    rank = np.empty(NPAD, dtype=np.int64)
    rank[perm] = np.arange(NPAD)
    degs = deg[perm]
    c_slots = tuple(
        int(max(1, degs[(NCORES * t) * P])) for t in range(TPC)
    )
    offs = np.concatenate([[0], np.cumsum(c_slots)]).astype(np.int64)
    sumc = int(offs[-1])

    # per-dst contiguous edge runs
    order = np.argsort(dst_all, kind="stable")
    src_s = src_all[order].astype(np.int32)
    dst_s = dst_all[order]
    starts = np.concatenate([[0], np.cumsum(np.bincount(dst_all, minlength=NPAD))])
    j = np.arange(dst_s.size, dtype=np.int64) - starts[dst_s]

    r = rank[dst_s]
    rt = r // P
    core = rt % NCORES
    slot = rt // NCORES
    p = r % P
    col = offs[slot] + j

    idx_full = np.full((NCORES, P, sumc), N, dtype=np.int32)  # pad -> zero row
    idx_full[core, p, col] = src_s

    # per-core dis / w vectors in (partition, slot) layout
    node_of = perm.reshape(TILES, P)  # [tile rank, partition] -> node
    disv_full = np.empty((NCORES, P, TPC), dtype=np.float32)
    wv_full = np.empty((NCORES, P, TPC), dtype=np.float16)
    for k in range(NCORES):
        sel = node_of[k::NCORES, :]  # [TPC, P]
        disv_full[k] = dis[sel].T.astype(np.float32)
        wv_full[k] = w[sel].T.astype(np.float16)

    xts = np.zeros((P, NPAD), dtype=np.float16)
    xts[:, :N] = (np.asarray(x, dtype=np.float64) * dis[:N, None]).T

    return c_slots, idx_full, disv_full, wv_full, xts


def _make_in_maps(inputs):
    c_slots, idx_full, disv_full, wv_full, xts = _prep(
        inputs["x"], inputs["edge_index"]
    )
    w1_d = np.asarray(inputs["W1"], dtype=np.float16)
    b1b = np.broadcast_to(
        np.asarray(inputs["b1"], dtype=np.float32), (P, D)
    ).copy()
    in_maps = []
    for k in range(NCORES):
        in_maps.append(
            {
                "xt": xts,
                "w1": w1_d,
                "idx": np.ascontiguousarray(idx_full[k]),
                "disv": np.ascontiguousarray(disv_full[k]),
                "wv": np.ascontiguousarray(wv_full[k]),
                "b1b": b1b,
            }
        )
    return c_slots, in_maps


def _run(inputs, trace=False):
    c_slots, in_maps = _make_in_maps(inputs)
    if c_slots not in _compiled:
        _compiled[c_slots] = _build(c_slots)
    nc = _compiled[c_slots]

    res = run_bass_kernel_spmd(
        nc, in_maps, core_ids=list(range(NCORES)), trace=trace
    )
    s_total = np.zeros(D, dtype=np.float64)
    for r in res.results:
        s_total += r["sout"][0].astype(np.float64)

    out = (s_total / N) @ np.asarray(inputs["W2"], dtype=np.float64) + np.asarray(
        inputs["b2"], dtype=np.float64
    )
    return out[None, :].astype(np.float32), res.exec_time_ns


def kernel(x, edge_index, W1, b1, W2, b2):
    out, _ = _run(
        {
            "x": x,
            "edge_index": edge_index,
            "W1": W1,
            "b1": b1,
            "W2": W2,
            "b2": b2,
        }
    )
    return out



# revision 2
# speedup vs baseline: 1.1164x; 1.1164x over previous
"""2-layer GCN (GCNConv -> relu -> GCNConv -> mean) on 8 trn2 NeuronCores.

v2: matmul moved AFTER aggregation (linearity), phase 1 eliminated.

Math restructure:
  reference output = mean_n(h2[n]) with h2 = A_norm @ (h1 @ W2) + b2,
  h1 = relu(A_norm @ (x @ W1) + b1), A_norm = D^-1/2 (A+I) D^-1/2.
  mean is linear -> layer 2 collapses to a weighted sum over h1 rows:
    mean(h2) = (1/N) * (sum_n w_n * h1[n]) @ W2 + b2,
    w_n = dis_n * sum_{e: src_e = n} dis_{dst_e}
  and A_norm @ (x @ W1) = (A_norm @ x) @ W1, so message passing runs on the
  RAW (dis-scaled) x rows in fp8 and the W1 matmul happens per dst tile
  after aggregation:
    h1[n] = relu((dis_n * agg_n) @ W1 + b1), agg_n = sum_{e->n} dis_src*x_src

Device work per core (SPMD, same program, different data):
  dst nodes degree-sorted, assigned (core, slot, partition) as rank r ->
  core (r//P)%8, slot (r//P)//8, partition r%P. Per slot:
    1. one indirect-DMA gather lands each dst's edge-source xq rows (fp8,
       128B each) in its partition,
    2. DVE pairwise tree -> agg [128, 128] fp16,
    3. *dis_dst (tensor_scalar), transpose via PE identity matmul,
    4. h1 = aggT^T @ W1 + b1, relu,
    5. s += w_slot^T @ o1 into persistent [1,128] PSUM accumulator.
  host: sum 8 partials, /N, @W2, +b2.
"""

import sys

sys.path.insert(0, "/opt/trn_rl_repo")

from contextlib import ExitStack

import ml_dtypes
import numpy as np

import concourse.tile as tile
from concourse import bacc, bass, mybir
from concourse.bass import IndirectOffsetOnAxis
from concourse.bass_utils import run_bass_kernel_spmd

N = 50000
P = 128
NCORES = 8
TILES = 392            # ceil(N / P) rounded up to a multiple of NCORES
NPAD = TILES * P       # 50176
TPC = TILES // NCORES  # 49 dst tiles (slots) per core
D = 128                # feature dim (in & hidden)

F8 = mybir.dt.float8e4
F16 = mybir.dt.float16
F32 = mybir.dt.float32
I32 = mybir.dt.int32

CAST_GATHER = True  # fp8 DRAM -> fp16 SBUF cast during gather

_compiled = {}


def _build(c_slots, cast_gather=CAST_GATHER):
    """Build the Bass program. c_slots[t] = edge K-capacity of slot t."""
    sumc = int(sum(c_slots))
    cmax = int(max(c_slots))
    nc = bacc.Bacc(
        "TRN2", target_bir_lowering=False, debug=False, num_devices=NCORES
    )
    xq = nc.declare_dram_parameter("xq", [NPAD, D], F8, isOutput=False)
    w1 = nc.declare_dram_parameter("w1", [P, D], F16, isOutput=False)
    idx = nc.declare_dram_parameter("idx", [P, sumc], I32, isOutput=False)
    disv = nc.declare_dram_parameter("disv", [P, TPC], F32, isOutput=False)
    wv = nc.declare_dram_parameter("wv", [P, TPC], F16, isOutput=False)
    b1b = nc.declare_dram_parameter("b1b", [P, D], F32, isOutput=False)
    ident = nc.declare_dram_parameter("ident", [P, D], F16, isOutput=False)
    sout = nc.declare_dram_parameter("sout", [1, D], F32, isOutput=True)

    with tile.TileContext(nc) as tc, ExitStack() as ctx:
        const = ctx.enter_context(tc.tile_pool(name="const", bufs=1))
        gpool = ctx.enter_context(tc.tile_pool(name="gather", bufs=4))
        tpool = ctx.enter_context(tc.tile_pool(name="tree", bufs=4))
        psT = ctx.enter_context(tc.tile_pool(name="psT", bufs=2, space="PSUM"))
        ps1 = ctx.enter_context(tc.tile_pool(name="ps1", bufs=2, space="PSUM"))
        sps = ctx.enter_context(tc.tile_pool(name="sps", bufs=1, space="PSUM"))
        epool = ctx.enter_context(tc.tile_pool(name="epi", bufs=3))
        opool = ctx.enter_context(tc.tile_pool(name="outp", bufs=1))

        # ---- resident constants ----
        w1_sb = const.tile([P, D], F16)
        nc.sync.dma_start(out=w1_sb[:], in_=w1[:])
        b1b_sb = const.tile([P, D], F32)
        nc.sync.dma_start(out=b1b_sb[:], in_=b1b[:])
        ident_sb = const.tile([P, D], F16)
        nc.sync.dma_start(out=ident_sb[:], in_=ident[:])
        disv_sb = const.tile([P, TPC], F32)
        nc.sync.dma_start(out=disv_sb[:], in_=disv[:])
        wv_sb = const.tile([P, TPC], F16)
        nc.sync.dma_start(out=wv_sb[:], in_=wv[:])
        idx_sb = const.tile([P, sumc], I32)
        nc.sync.dma_start(out=idx_sb[:], in_=idx[:])

        s_ps = sps.tile([1, D], F32)
        gdt = F16 if cast_gather else F8
        off = 0
        for t in range(TPC):
            c = int(c_slots[t])
            gath = gpool.tile([P, cmax * D], gdt, tag="gather")
            nc.gpsimd.indirect_dma_start(
                out=gath[:, : c * D],
                out_offset=None,
                in_=xq[:],
                in_offset=IndirectOffsetOnAxis(
                    ap=idx_sb[:, off : off + c], axis=0
                ),
            )
            # ---- segment sum -> agg_ap [P, D] fp16 ----
            if cast_gather:
                cur = c
                while cur > 1:
                    h = cur // 2
                    r = cur - h
                    nc.vector.tensor_tensor(
                        out=gath[:, : h * D],
                        in0=gath[:, : h * D],
                        in1=gath[:, r * D : cur * D],
                        op=mybir.AluOpType.add,
                    )
                    cur = r
                agg_ap = gath[:, :D]
            else:
                tree = tpool.tile([P, max(1, (cmax + 1) // 2) * D], F16, tag="tree")
                if c == 1:
                    nc.vector.tensor_copy(out=tree[:, :D], in_=gath[:, :D])
                else:
                    h = c // 2
                    r = c - h
                    # level 1: fp8 + fp8 -> fp16
                    nc.vector.tensor_tensor(
                        out=tree[:, : h * D],
                        in0=gath[:, : h * D],
                        in1=gath[:, r * D : c * D],
                        op=mybir.AluOpType.add,
                    )
                    if r > h:
                        nc.vector.tensor_copy(
                            out=tree[:, h * D : r * D],
                            in_=gath[:, h * D : r * D],
                        )
                    cur = r
                    while cur > 1:
                        h2 = cur // 2
                        r2 = cur - h2
                        nc.vector.tensor_tensor(
                            out=tree[:, : h2 * D],
                            in0=tree[:, : h2 * D],
                            in1=tree[:, r2 * D : cur * D],
                            op=mybir.AluOpType.add,
                        )
                        cur = r2
                agg_ap = tree[:, :D]

            # ---- epilogue ----
            agg16 = epool.tile([P, D], F16, tag="agg16")
            nc.vector.tensor_scalar_mul(agg16[:], agg_ap, disv_sb[:, t : t + 1])
            pT = psT.tile([P, D], F32, tag="pT")
            nc.tensor.matmul(
                out=pT[:], lhsT=agg16[:], rhs=ident_sb[:], start=True, stop=True
            )
            aggT = epool.tile([P, D], F16, tag="aggT")
            nc.scalar.activation(
                out=aggT[:], in_=pT[:], func=mybir.ActivationFunctionType.Copy
            )
            p1 = ps1.tile([P, D], F32, tag="p1")
            nc.tensor.matmul(
                out=p1[:], lhsT=aggT[:], rhs=w1_sb[:], start=True, stop=True
            )
            h1b = epool.tile([P, D], F32, tag="h1b")
            nc.vector.tensor_tensor(
                out=h1b[:], in0=p1[:], in1=b1b_sb[:], op=mybir.AluOpType.add
            )
            o1 = epool.tile([P, D], F16, tag="o1")
            nc.scalar.activation(
                out=o1[:], in_=h1b[:], func=mybir.ActivationFunctionType.Relu
            )
            nc.tensor.matmul(
                out=s_ps[:],
                lhsT=wv_sb[:, t : t + 1],
                rhs=o1[:],
                start=(t == 0),
                stop=(t == TPC - 1),
                skip_group_check=True,
            )
            off += c

        s_sb = opool.tile([1, D], F32)
        nc.vector.tensor_copy(out=s_sb[:], in_=s_ps[:])
        nc.sync.dma_start(out=sout[:], in_=s_sb[:])

    nc.compile()
    return nc


def _prep(x, edge_index):
    """Host-side graph preprocessing -> per-core device input maps."""
    src = np.asarray(edge_index[0], dtype=np.int64)
    dst = np.asarray(edge_index[1], dtype=np.int64)
    loop = np.arange(N, dtype=np.int64)
    src_all = np.concatenate([src, loop])
    dst_all = np.concatenate([dst, loop])

    deg = np.bincount(dst_all, minlength=NPAD).astype(np.int64)
    dis = np.zeros(NPAD, dtype=np.float64)
    nz = deg > 0
    dis[nz] = 1.0 / np.sqrt(deg[nz])

    acc = np.zeros(NPAD, dtype=np.float64)
    np.add.at(acc, src_all, dis[dst_all])
    w = dis * acc  # layer-2 collapsed per-node weight

    # degree-sorted relabeling: rank r -> node perm[r];
    # tile rank rt = r // P -> core rt % 8, slot rt // 8, partition r % P.
    perm = np.argsort(-deg, kind="stable")
    rank = np.empty(NPAD, dtype=np.int64)
    rank[perm] = np.arange(NPAD)
    degs = deg[perm]
    c_slots = tuple(
        int(max(1, degs[(NCORES * t) * P])) for t in range(TPC)
    )
    offs = np.concatenate([[0], np.cumsum(c_slots)]).astype(np.int64)
    sumc = int(offs[-1])

    # per-dst contiguous edge runs
    order = np.argsort(dst_all, kind="stable")
    src_s = src_all[order].astype(np.int32)
    dst_s = dst_all[order]
    starts = np.concatenate([[0], np.cumsum(np.bincount(dst_all, minlength=NPAD))])
    j = np.arange(dst_s.size, dtype=np.int64) - starts[dst_s]

    r = rank[dst_s]
    rt = r // P
    core = rt % NCORES
    slot = rt // NCORES
    p = r % P
    col = offs[slot] + j

    idx_full = np.full((NCORES, P, sumc), N, dtype=np.int32)  # pad -> zero row
    idx_full[core, p, col] = src_s

    # per-core dis / w vectors in (partition, slot) layout
    node_of = perm.reshape(TILES, P)  # [tile rank, partition] -> node
    disv_full = np.empty((NCORES, P, TPC), dtype=np.float32)
    wv_full = np.empty((NCORES, P, TPC), dtype=np.float16)
    for k in range(NCORES):
        sel = node_of[k::NCORES, :]  # [TPC, P]
        disv_full[k] = dis[sel].T.astype(np.float32)
        wv_full[k] = w[sel].T.astype(np.float16)

    # xq[n] = dis_n * x_n, quantized to fp8 e4m3 (TRN max normal 240)
    xq = np.zeros((NPAD, D), dtype=ml_dtypes.float8_e4m3)
    xv = np.asarray(x, dtype=np.float64) * dis[:N, None]
    xq[:N] = np.clip(xv, -240, 240).astype(ml_dtypes.float8_e4m3)

    return c_slots, idx_full, disv_full, wv_full, xq


def _make_in_maps(inputs):
    c_slots, idx_full, disv_full, wv_full, xq = _prep(
        inputs["x"], inputs["edge_index"]
    )
    w1_d = np.asarray(inputs["W1"], dtype=np.float16)
    b1b = np.broadcast_to(
        np.asarray(inputs["b1"], dtype=np.float32), (P, D)
    ).copy()
    ident = np.eye(P, D, dtype=np.float16)
    in_maps = []
    for k in range(NCORES):
        in_maps.append(
            {
                "xq": xq,
                "w1": w1_d,
                "idx": np.ascontiguousarray(idx_full[k]),
                "disv": np.ascontiguousarray(disv_full[k]),
                "wv": np.ascontiguousarray(wv_full[k]),
                "b1b": b1b,
                "ident": ident,
            }
        )
    return c_slots, in_maps


def _run(inputs, trace=False):
    c_slots, in_maps = _make_in_maps(inputs)
    if c_slots not in _compiled:
        _compiled[c_slots] = _build(c_slots)
    nc = _compiled[c_slots]

    res = run_bass_kernel_spmd(
        nc, in_maps, core_ids=list(range(NCORES)), trace=trace
    )
    s_total = np.zeros(D, dtype=np.float64)
    for r in res.results:
        s_total += r["sout"][0].astype(np.float64)

    out = (s_total / N) @ np.asarray(inputs["W2"], dtype=np.float64) + np.asarray(
        inputs["b2"], dtype=np.float64
    )
    return out[None, :].astype(np.float32), res.exec_time_ns


def kernel(x, edge_index, W1, b1, W2, b2):
    out, _ = _run(
        {
            "x": x,
            "edge_index": edge_index,
            "W1": W1,
            "b1": b1,
            "W2": W2,
            "b2": b2,
        }
    )
    return out


# revision 4
# speedup vs baseline: 1.2821x; 1.1485x over previous
"""2-layer GCN (GCNConv -> relu -> GCNConv -> mean) on 8 trn2 NeuronCores.

W1 matmul moved AFTER aggregation (linearity), so no dense phase over all
nodes is needed: per-core work is one fp8 gather + fp16 tree + small
matmuls per 128-dst slot. b1 is folded into the PE as a 1-row matmul
accumulating ones^T @ b1 into the same PSUM tile (keeps DVE off the
critical path).

Math restructure:
  reference output = mean_n(h2[n]) with h2 = A_norm @ (h1 @ W2) + b2,
  h1 = relu(A_norm @ (x @ W1) + b1), A_norm = D^-1/2 (A+I) D^-1/2.
  mean is linear -> layer 2 collapses to a weighted sum over h1 rows:
    mean(h2) = (1/N) * (sum_n w_n * h1[n]) @ W2 + b2,
    w_n = dis_n * sum_{e: src_e = n} dis_{dst_e}
  and A_norm @ (x @ W1) = (A_norm @ x) @ W1, so message passing runs on the
  RAW (dis-scaled) x rows in fp8 and the W1 matmul happens per dst tile
  after aggregation:
    h1[n] = relu((dis_n * agg_n) @ W1 + b1), agg_n = sum_{e->n} dis_src*x_src

Device work per core (SPMD, same program, different data):
  dst nodes degree-sorted, assigned (core, slot, partition) as rank r ->
  core (r//P)%8, slot (r//P)//8, partition r%P. Per slot:
    1. one indirect-DMA gather lands each dst's edge-source xq rows (fp8,
       128B each) in its partition,
    2. DVE pairwise tree -> agg [128, 128] fp16,
    3. *dis_dst (tensor_scalar), transpose via PE identity matmul,
    4. h1 = aggT^T @ W1 + b1, relu,
    5. s += w_slot^T @ o1 into persistent [1,128] PSUM accumulator.
  host: sum 8 partials, /N, @W2, +b2.
"""

import sys

sys.path.insert(0, "/opt/trn_rl_repo")

from contextlib import ExitStack

import ml_dtypes
import numpy as np

import concourse.tile as tile
from concourse import bacc, bass, mybir
from concourse.bass import IndirectOffsetOnAxis
from concourse.bass_utils import run_bass_kernel_spmd

N = 50000
P = 128
NCORES = 8
TILES = 392            # ceil(N / P) rounded up to a multiple of NCORES
NPAD = TILES * P       # 50176
TPC = TILES // NCORES  # 49 dst tiles (slots) per core
D = 128                # feature dim (in & hidden)

F8 = mybir.dt.float8e4
F16 = mybir.dt.float16
F32 = mybir.dt.float32
I32 = mybir.dt.int32

CAST_GATHER = True  # fp8 DRAM -> fp16 SBUF cast during gather

_compiled = {}


def _build(c_slots, cast_gather=CAST_GATHER):
    """Build the Bass program. c_slots[t] = edge K-capacity of slot t."""
    sumc = int(sum(c_slots))
    cmax = int(max(c_slots))
    nc = bacc.Bacc(
        "TRN2", target_bir_lowering=False, debug=False, num_devices=NCORES
    )
    xq = nc.declare_dram_parameter("xq", [NPAD, D], F8, isOutput=False)
    w1 = nc.declare_dram_parameter("w1", [P, D], F16, isOutput=False)
    idx = nc.declare_dram_parameter("idx", [P, sumc], I32, isOutput=False)
    disv = nc.declare_dram_parameter("disv", [P, TPC], F32, isOutput=False)
    wv = nc.declare_dram_parameter("wv", [P, TPC], F16, isOutput=False)
    b1r = nc.declare_dram_parameter("b1r", [1, D], F16, isOutput=False)
    ones = nc.declare_dram_parameter("ones", [1, D], F16, isOutput=False)
    ident = nc.declare_dram_parameter("ident", [P, D], F16, isOutput=False)
    sout = nc.declare_dram_parameter("sout", [1, D], F32, isOutput=True)

    with tile.TileContext(nc) as tc, ExitStack() as ctx:
        const = ctx.enter_context(tc.tile_pool(name="const", bufs=1))
        gpool = ctx.enter_context(tc.tile_pool(name="gather", bufs=4))
        tpool = ctx.enter_context(tc.tile_pool(name="tree", bufs=4))
        psT = ctx.enter_context(tc.tile_pool(name="psT", bufs=2, space="PSUM"))
        ps1 = ctx.enter_context(tc.tile_pool(name="ps1", bufs=2, space="PSUM"))
        sps = ctx.enter_context(tc.tile_pool(name="sps", bufs=1, space="PSUM"))
        epool = ctx.enter_context(tc.tile_pool(name="epi", bufs=3))
        opool = ctx.enter_context(tc.tile_pool(name="outp", bufs=1))

        # ---- resident constants ----
        w1_sb = const.tile([P, D], F16)
        nc.sync.dma_start(out=w1_sb[:], in_=w1[:])
        b1r_sb = const.tile([1, D], F16)
        nc.sync.dma_start(out=b1r_sb[:], in_=b1r[:])
        ones_sb = const.tile([1, D], F16)
        nc.sync.dma_start(out=ones_sb[:], in_=ones[:])
        ident_sb = const.tile([P, D], F16)
        nc.sync.dma_start(out=ident_sb[:], in_=ident[:])
        disv_sb = const.tile([P, TPC], F32)
        nc.sync.dma_start(out=disv_sb[:], in_=disv[:])
        wv_sb = const.tile([P, TPC], F16)
        nc.sync.dma_start(out=wv_sb[:], in_=wv[:])
        idx_sb = const.tile([P, sumc], I32)
        nc.sync.dma_start(out=idx_sb[:], in_=idx[:])

        s_ps = sps.tile([1, D], F32)
        gdt = F16 if cast_gather else F8
        off = 0
        for t in range(TPC):
            c = int(c_slots[t])
            gath = gpool.tile([P, cmax * D], gdt, tag="gather")
            nc.gpsimd.indirect_dma_start(
                out=gath[:, : c * D],
                out_offset=None,
                in_=xq[:],
                in_offset=IndirectOffsetOnAxis(
                    ap=idx_sb[:, off : off + c], axis=0
                ),
            )
            # ---- segment sum -> agg_ap [P, D] fp16 ----
            if cast_gather:
                cur = c
                while cur > 1:
                    h = cur // 2
                    r = cur - h
                    nc.vector.tensor_tensor(
                        out=gath[:, : h * D],
                        in0=gath[:, : h * D],
                        in1=gath[:, r * D : cur * D],
                        op=mybir.AluOpType.add,
                    )
                    cur = r
                agg_ap = gath[:, :D]
            else:
                tree = tpool.tile([P, max(1, (cmax + 1) // 2) * D], F16, tag="tree")
                if c == 1:
                    nc.vector.tensor_copy(out=tree[:, :D], in_=gath[:, :D])
                else:
                    h = c // 2
                    r = c - h
                    # level 1: fp8 + fp8 -> fp16
                    nc.vector.tensor_tensor(
                        out=tree[:, : h * D],
                        in0=gath[:, : h * D],
                        in1=gath[:, r * D : c * D],
                        op=mybir.AluOpType.add,
                    )
                    if r > h:
                        nc.vector.tensor_copy(
                            out=tree[:, h * D : r * D],
                            in_=gath[:, h * D : r * D],
                        )
                    cur = r
                    while cur > 1:
                        h2 = cur // 2
                        r2 = cur - h2
                        nc.vector.tensor_tensor(
                            out=tree[:, : h2 * D],
                            in0=tree[:, : h2 * D],
                            in1=tree[:, r2 * D : cur * D],
                            op=mybir.AluOpType.add,
                        )
                        cur = r2
                agg_ap = tree[:, :D]

            # ---- epilogue ----
            agg16 = epool.tile([P, D], F16, tag="agg16")
            nc.vector.tensor_scalar_mul(agg16[:], agg_ap, disv_sb[:, t : t + 1])
            pT = psT.tile([P, D], F32, tag="pT")
            nc.tensor.matmul(
                out=pT[:], lhsT=agg16[:], rhs=ident_sb[:], start=True, stop=True
            )
            aggT = epool.tile([P, D], F16, tag="aggT")
            nc.scalar.activation(
                out=aggT[:], in_=pT[:], func=mybir.ActivationFunctionType.Copy
            )
            p1 = ps1.tile([P, D], F32, tag="p1")
            nc.tensor.matmul(
                out=p1[:], lhsT=aggT[:], rhs=w1_sb[:], start=True, stop=False
            )
            # += ones^T @ b1 : adds b1[f] to every dst row (bias fold on PE)
            nc.tensor.matmul(
                out=p1[:], lhsT=ones_sb[:], rhs=b1r_sb[:], start=False, stop=True
            )
            o1 = epool.tile([P, D], F16, tag="o1")
            nc.scalar.activation(
                out=o1[:], in_=p1[:], func=mybir.ActivationFunctionType.Relu
            )
            nc.tensor.matmul(
                out=s_ps[:],
                lhsT=wv_sb[:, t : t + 1],
                rhs=o1[:],
                start=(t == 0),
                stop=(t == TPC - 1),
                skip_group_check=True,
            )
            off += c

        s_sb = opool.tile([1, D], F32)
        nc.vector.tensor_copy(out=s_sb[:], in_=s_ps[:])
        nc.sync.dma_start(out=sout[:], in_=s_sb[:])

    nc.compile()
    return nc


def _prep(x, edge_index):
    """Host-side graph preprocessing -> per-core device input maps."""
    src = np.asarray(edge_index[0], dtype=np.int64)
    dst = np.asarray(edge_index[1], dtype=np.int64)
    loop = np.arange(N, dtype=np.int64)
    src_all = np.concatenate([src, loop])
    dst_all = np.concatenate([dst, loop])

    deg = np.bincount(dst_all, minlength=NPAD).astype(np.int64)
    dis = np.zeros(NPAD, dtype=np.float64)
    nz = deg > 0
    dis[nz] = 1.0 / np.sqrt(deg[nz])

    acc = np.zeros(NPAD, dtype=np.float64)
    np.add.at(acc, src_all, dis[dst_all])
    w = dis * acc  # layer-2 collapsed per-node weight

    # degree-sorted relabeling: rank r -> node perm[r];
    # tile rank rt = r // P -> core rt % 8, slot rt // 8, partition r % P.
    perm = np.argsort(-deg, kind="stable")
    rank = np.empty(NPAD, dtype=np.int64)
    rank[perm] = np.arange(NPAD)
    degs = deg[perm]
    c_slots = tuple(
        int(max(1, degs[(NCORES * t) * P])) for t in range(TPC)
    )
    offs = np.concatenate([[0], np.cumsum(c_slots)]).astype(np.int64)
    sumc = int(offs[-1])

    # per-dst contiguous edge runs
    order = np.argsort(dst_all, kind="stable")
    src_s = src_all[order].astype(np.int32)
    dst_s = dst_all[order]
    starts = np.concatenate([[0], np.cumsum(np.bincount(dst_all, minlength=NPAD))])
    j = np.arange(dst_s.size, dtype=np.int64) - starts[dst_s]

    r = rank[dst_s]
    rt = r // P
    core = rt % NCORES
    slot = rt // NCORES
    p = r % P
    col = offs[slot] + j

    idx_full = np.full((NCORES, P, sumc), N, dtype=np.int32)  # pad -> zero row
    idx_full[core, p, col] = src_s

    # per-core dis / w vectors in (partition, slot) layout
    node_of = perm.reshape(TILES, P)  # [tile rank, partition] -> node
    disv_full = np.empty((NCORES, P, TPC), dtype=np.float32)
    wv_full = np.empty((NCORES, P, TPC), dtype=np.float16)
    for k in range(NCORES):
        sel = node_of[k::NCORES, :]  # [TPC, P]
        disv_full[k] = dis[sel].T.astype(np.float32)
        wv_full[k] = w[sel].T.astype(np.float16)

    # xq[n] = dis_n * x_n, quantized to fp8 e4m3 (TRN max normal 240)
    xq = np.zeros((NPAD, D), dtype=ml_dtypes.float8_e4m3)
    xv = np.asarray(x, dtype=np.float64) * dis[:N, None]
    xq[:N] = np.clip(xv, -240, 240).astype(ml_dtypes.float8_e4m3)

    return c_slots, idx_full, disv_full, wv_full, xq


def _make_in_maps(inputs):
    c_slots, idx_full, disv_full, wv_full, xq = _prep(
        inputs["x"], inputs["edge_index"]
    )
    w1_d = np.asarray(inputs["W1"], dtype=np.float16)
    b1r = np.asarray(inputs["b1"], dtype=np.float16).reshape(1, D)
    ones = np.ones((1, D), dtype=np.float16)
    ident = np.eye(P, D, dtype=np.float16)
    in_maps = []
    for k in range(NCORES):
        in_maps.append(
            {
                "xq": xq,
                "w1": w1_d,
                "idx": np.ascontiguousarray(idx_full[k]),
                "disv": np.ascontiguousarray(disv_full[k]),
                "wv": np.ascontiguousarray(wv_full[k]),
                "b1r": b1r,
                "ones": ones,
                "ident": ident,
            }
        )
    return c_slots, in_maps


def _run(inputs, trace=False):
    c_slots, in_maps = _make_in_maps(inputs)
    if c_slots not in _compiled:
        _compiled[c_slots] = _build(c_slots)
    nc = _compiled[c_slots]

    res = run_bass_kernel_spmd(
        nc, in_maps, core_ids=list(range(NCORES)), trace=trace
    )
    s_total = np.zeros(D, dtype=np.float64)
    for r in res.results:
        s_total += r["sout"][0].astype(np.float64)

    out = (s_total / N) @ np.asarray(inputs["W2"], dtype=np.float64) + np.asarray(
        inputs["b2"], dtype=np.float64
    )
    return out[None, :].astype(np.float32), res.exec_time_ns


def kernel(x, edge_index, W1, b1, W2, b2):
    out, _ = _run(
        {
            "x": x,
            "edge_index": edge_index,
            "W1": W1,
            "b1": b1,
            "W2": W2,
            "b2": b2,
        }
    )
    return out


# revision 5
# speedup vs baseline: 1.4524x; 1.1328x over previous
"""2-layer GCN (GCNConv -> relu -> GCNConv -> mean) on 8 trn2 NeuronCores.

v2: matmul moved AFTER aggregation (linearity), phase 1 eliminated.

Math restructure:
  reference output = mean_n(h2[n]) with h2 = A_norm @ (h1 @ W2) + b2,
  h1 = relu(A_norm @ (x @ W1) + b1), A_norm = D^-1/2 (A+I) D^-1/2.
  mean is linear -> layer 2 collapses to a weighted sum over h1 rows:
    mean(h2) = (1/N) * (sum_n w_n * h1[n]) @ W2 + b2,
    w_n = dis_n * sum_{e: src_e = n} dis_{dst_e}
  and A_norm @ (x @ W1) = (A_norm @ x) @ W1, so message passing runs on the
  RAW (dis-scaled) x rows in fp8 and the W1 matmul happens per dst tile
  after aggregation:
    h1[n] = relu((dis_n * agg_n) @ W1 + b1), agg_n = sum_{e->n} dis_src*x_src

Device work per core (SPMD, same program, different data):
  dst nodes degree-sorted, assigned (core, slot, partition) as rank r ->
  core (r//P)%8, slot (r//P)//8, partition r%P. Per slot:
    1. one indirect-DMA gather lands each dst's edge-source xq rows (fp8,
       128B each) in its partition,
    2. DVE pairwise tree -> agg [128, 128] fp16,
    3. *dis_dst (tensor_scalar), transpose via PE identity matmul,
    4. h1 = aggT^T @ W1 + b1, relu,
    5. s += w_slot^T @ o1 into persistent [1,128] PSUM accumulator.
  host: sum 8 partials, /N, @W2, +b2.
"""

import sys

sys.path.insert(0, "/opt/trn_rl_repo")

from contextlib import ExitStack

import ml_dtypes
import numpy as np

import concourse.tile as tile
from concourse import bacc, bass, mybir
from concourse.bass import IndirectOffsetOnAxis
from concourse.bass_utils import run_bass_kernel_spmd

N = 50000
P = 128
NCORES = 8
TILES = 392            # ceil(N / P) rounded up to a multiple of NCORES
NPAD = TILES * P       # 50176
TPC = TILES // NCORES  # 49 dst tiles (slots) per core
D = 128                # feature dim (in & hidden)

F8 = mybir.dt.float8e4
F16 = mybir.dt.float16
F32 = mybir.dt.float32
I32 = mybir.dt.int32

CAST_GATHER = True  # fp8 DRAM -> fp16 SBUF cast during gather

_compiled = {}


def _groups(c_slots):
    """Pair adjacent slots (padded to a common capacity) + odd leftover."""
    gs = []
    u = 0
    while u + 1 < TPC:
        gs.append((u, u + 1))
        u += 2
    if u < TPC:
        gs.append((u,))
    widths = [
        len(g) * max(int(c_slots[t]) for t in g) for g in gs
    ]
    return gs, widths


def _build(c_slots, cast_gather=CAST_GATHER):
    """Build the Bass program. c_slots[t] = edge K-capacity of slot t."""
    groups, widths = _groups(c_slots)
    sumc = int(sum(widths))
    split = int(sum(widths[:3]))  # first 3 groups load first (fast start)
    wmax = int(max(widths))
    nc = bacc.Bacc(
        "TRN2", target_bir_lowering=False, debug=False, num_devices=NCORES
    )
    xq = nc.declare_dram_parameter("xq", [NPAD, D], F8, isOutput=False)
    w1 = nc.declare_dram_parameter("w1", [P, D], F16, isOutput=False)
    idxa = nc.declare_dram_parameter("idxa", [P, split], I32, isOutput=False)
    idxb = nc.declare_dram_parameter("idxb", [P, sumc - split], I32, isOutput=False)
    disv = nc.declare_dram_parameter("disv", [P, TPC], F32, isOutput=False)
    wv = nc.declare_dram_parameter("wv", [P, TPC], F16, isOutput=False)
    b1r = nc.declare_dram_parameter("b1r", [1, D], F16, isOutput=False)
    ones = nc.declare_dram_parameter("ones", [1, D], F16, isOutput=False)
    ident = nc.declare_dram_parameter("ident", [P, D], F16, isOutput=False)
    sout = nc.declare_dram_parameter("sout", [1, D], F32, isOutput=True)

    with tile.TileContext(nc) as tc, ExitStack() as ctx:
        const = ctx.enter_context(tc.tile_pool(name="const", bufs=1))
        gpool = ctx.enter_context(tc.tile_pool(name="gather", bufs=4))
        tpool = ctx.enter_context(tc.tile_pool(name="tree", bufs=4))
        psT = ctx.enter_context(tc.tile_pool(name="psT", bufs=2, space="PSUM"))
        ps1 = ctx.enter_context(tc.tile_pool(name="ps1", bufs=2, space="PSUM"))
        sps = ctx.enter_context(tc.tile_pool(name="sps", bufs=1, space="PSUM"))
        epool = ctx.enter_context(tc.tile_pool(name="epi", bufs=3))
        opool = ctx.enter_context(tc.tile_pool(name="outp", bufs=1))

        # ---- resident constants; idxa first so slot-0 gather starts early ----
        idxa_sb = const.tile([P, split], I32)
        nc.sync.dma_start(out=idxa_sb[:], in_=idxa[:])
        w1_sb = const.tile([P, D], F16)
        nc.sync.dma_start(out=w1_sb[:], in_=w1[:])
        b1r_sb = const.tile([1, D], F16)
        nc.sync.dma_start(out=b1r_sb[:], in_=b1r[:])
        ones_sb = const.tile([1, D], F16)
        nc.sync.dma_start(out=ones_sb[:], in_=ones[:])
        ident_sb = const.tile([P, D], F16)
        nc.sync.dma_start(out=ident_sb[:], in_=ident[:])
        disv_sb = const.tile([P, TPC], F32)
        nc.sync.dma_start(out=disv_sb[:], in_=disv[:])
        wv_sb = const.tile([P, TPC], F16)
        nc.sync.dma_start(out=wv_sb[:], in_=wv[:])
        idxb_sb = const.tile([P, sumc - split], I32)
        nc.sync.dma_start(out=idxb_sb[:], in_=idxb[:])

        s_ps = sps.tile([1, D], F32)
        off = 0
        for gi, g in enumerate(groups):
            width = widths[gi]
            cp = width // len(g)
            src_sb, src_off = (
                (idxa_sb, off) if off + width <= split else (idxb_sb, off - split)
            )
            gath = gpool.tile([P, wmax * D], F16, tag="gather")
            nc.gpsimd.indirect_dma_start(
                out=gath[:, : width * D],
                out_offset=None,
                in_=xq[:],
                in_offset=IndirectOffsetOnAxis(
                    ap=src_sb[:, src_off : src_off + width], axis=0
                ),
            )
            # ---- segment sum: shared pairwise tree over the group ----
            if cp > 1:
                if len(g) == 2:
                    gv = gath[:, : width * D].rearrange(
                        "p (s e f) -> p s e f", s=2, e=cp
                    )
                    cur = cp
                    while cur > 1:
                        h = cur // 2
                        r = cur - h
                        nc.vector.tensor_tensor(
                            out=gv[:, :, 0:h, :],
                            in0=gv[:, :, 0:h, :],
                            in1=gv[:, :, r:cur, :],
                            op=mybir.AluOpType.add,
                        )
                        cur = r
                else:
                    cur = cp
                    while cur > 1:
                        h = cur // 2
                        r = cur - h
                        nc.vector.tensor_tensor(
                            out=gath[:, : h * D],
                            in0=gath[:, : h * D],
                            in1=gath[:, r * D : cur * D],
                            op=mybir.AluOpType.add,
                        )
                        cur = r

            # ---- epilogue per slot in the group ----
            for si, t in enumerate(g):
                agg_ap = gath[:, si * cp * D : si * cp * D + D]
                agg16 = epool.tile([P, D], F16, tag="agg16")
                nc.vector.tensor_scalar_mul(
                    agg16[:], agg_ap, disv_sb[:, t : t + 1]
                )
                pT = psT.tile([P, D], F32, tag="pT")
                nc.tensor.matmul(
                    out=pT[:], lhsT=agg16[:], rhs=ident_sb[:],
                    start=True, stop=True,
                )
                aggT = epool.tile([P, D], F16, tag="aggT")
                nc.scalar.activation(
                    out=aggT[:], in_=pT[:],
                    func=mybir.ActivationFunctionType.Copy,
                )
                p1 = ps1.tile([P, D], F32, tag="p1")
                nc.tensor.matmul(
                    out=p1[:], lhsT=aggT[:], rhs=w1_sb[:],
                    start=True, stop=False,
                )
                # += ones^T @ b1 (bias fold on PE)
                nc.tensor.matmul(
                    out=p1[:], lhsT=ones_sb[:], rhs=b1r_sb[:],
                    start=False, stop=True,
                )
                o1 = epool.tile([P, D], F16, tag="o1")
                nc.scalar.activation(
                    out=o1[:], in_=p1[:],
                    func=mybir.ActivationFunctionType.Relu,
                )
                nc.tensor.matmul(
                    out=s_ps[:],
                    lhsT=wv_sb[:, t : t + 1],
                    rhs=o1[:],
                    start=(t == 0),
                    stop=(t == TPC - 1),
                    skip_group_check=True,
                )
            off += width

        s_sb = opool.tile([1, D], F32)
        nc.vector.tensor_copy(out=s_sb[:], in_=s_ps[:])
        nc.sync.dma_start(out=sout[:], in_=s_sb[:])

    nc.compile()
    return nc


def _prep(x, edge_index):
    """Host-side graph preprocessing -> per-core device input maps."""
    src = np.asarray(edge_index[0], dtype=np.int64)
    dst = np.asarray(edge_index[1], dtype=np.int64)
    loop = np.arange(N, dtype=np.int64)
    src_all = np.concatenate([src, loop])
    dst_all = np.concatenate([dst, loop])

    deg = np.bincount(dst_all, minlength=NPAD).astype(np.int64)
    dis = np.zeros(NPAD, dtype=np.float64)
    nz = deg > 0
    dis[nz] = 1.0 / np.sqrt(deg[nz])

    acc = np.zeros(NPAD, dtype=np.float64)
    np.add.at(acc, src_all, dis[dst_all])
    w = dis * acc  # layer-2 collapsed per-node weight

    # degree-sorted relabeling: rank r -> node perm[r];
    # tile rank rt = r // P -> core rt % 8, slot rt // 8, partition r % P.
    perm = np.argsort(-deg, kind="stable")
    rank = np.empty(NPAD, dtype=np.int64)
    rank[perm] = np.arange(NPAD)
    degs = deg[perm]
    c_slots = tuple(
        int(max(1, degs[(NCORES * t) * P])) for t in range(TPC)
    )
    groups, widths = _groups(c_slots)
    goffs = np.concatenate([[0], np.cumsum(widths)]).astype(np.int64)
    sumc = int(goffs[-1])
    col_base = np.zeros(TPC, dtype=np.int64)
    for gi, g in enumerate(groups):
        cp = widths[gi] // len(g)
        for si, t in enumerate(g):
            col_base[t] = goffs[gi] + si * cp

    # per-dst contiguous edge runs
    order = np.argsort(dst_all, kind="stable")
    src_s = src_all[order].astype(np.int32)
    dst_s = dst_all[order]
    starts = np.concatenate([[0], np.cumsum(np.bincount(dst_all, minlength=NPAD))])
    j = np.arange(dst_s.size, dtype=np.int64) - starts[dst_s]

    r = rank[dst_s]
    rt = r // P
    core = rt % NCORES
    slot = rt // NCORES
    p = r % P
    col = col_base[slot] + j

    idx_full = np.full((NCORES, P, sumc), N, dtype=np.int32)  # pad -> zero row
    idx_full[core, p, col] = src_s

    # per-core dis / w vectors in (partition, slot) layout
    node_of = perm.reshape(TILES, P)  # [tile rank, partition] -> node
    disv_full = np.empty((NCORES, P, TPC), dtype=np.float32)
    wv_full = np.empty((NCORES, P, TPC), dtype=np.float16)
    for k in range(NCORES):
        sel = node_of[k::NCORES, :]  # [TPC, P]
        disv_full[k] = dis[sel].T.astype(np.float32)
        wv_full[k] = w[sel].T.astype(np.float16)

    # xq[n] = dis_n * x_n, quantized to fp8 e4m3 (TRN max normal 240)
    xq = np.zeros((NPAD, D), dtype=ml_dtypes.float8_e4m3)
    xv = np.asarray(x, dtype=np.float64) * dis[:N, None]
    xq[:N] = np.clip(xv, -240, 240).astype(ml_dtypes.float8_e4m3)

    return c_slots, idx_full, disv_full, wv_full, xq


def _make_in_maps(inputs):
    c_slots, idx_full, disv_full, wv_full, xq = _prep(
        inputs["x"], inputs["edge_index"]
    )
    _, widths = _groups(c_slots)
    _split = int(sum(widths[:3]))
    w1_d = np.asarray(inputs["W1"], dtype=np.float16)
    b1r = np.asarray(inputs["b1"], dtype=np.float16).reshape(1, D)
    ones = np.ones((1, D), dtype=np.float16)
    ident = np.eye(P, D, dtype=np.float16)
    in_maps = []
    for k in range(NCORES):
        in_maps.append(
            {
                "xq": xq,
                "w1": w1_d,
                "idxa": np.ascontiguousarray(idx_full[k][:, :_split]),
                "idxb": np.ascontiguousarray(idx_full[k][:, _split:]),
                "disv": np.ascontiguousarray(disv_full[k]),
                "wv": np.ascontiguousarray(wv_full[k]),
                "b1r": b1r,
                "ones": ones,
                "ident": ident,
            }
        )
    return c_slots, in_maps


def _run(inputs, trace=False):
    c_slots, in_maps = _make_in_maps(inputs)
    if c_slots not in _compiled:
        _compiled[c_slots] = _build(c_slots)
    nc = _compiled[c_slots]

    res = run_bass_kernel_spmd(
        nc, in_maps, core_ids=list(range(NCORES)), trace=trace
    )
    s_total = np.zeros(D, dtype=np.float64)
    for r in res.results:
        s_total += r["sout"][0].astype(np.float64)

    out = (s_total / N) @ np.asarray(inputs["W2"], dtype=np.float64) + np.asarray(
        inputs["b2"], dtype=np.float64
    )
    return out[None, :].astype(np.float32), res.exec_time_ns


def kernel(x, edge_index, W1, b1, W2, b2):
    out, _ = _run(
        {
            "x": x,
            "edge_index": edge_index,
            "W1": W1,
            "b1": b1,
            "W2": W2,
            "b2": b2,
        }
    )
    return out


# revision 7
# speedup vs baseline: 1.4746x; 1.0153x over previous
"""2-layer GCN (GCNConv -> relu -> GCNConv -> mean) on 8 trn2 NeuronCores.

v2: matmul moved AFTER aggregation (linearity), phase 1 eliminated.

Math restructure:
  reference output = mean_n(h2[n]) with h2 = A_norm @ (h1 @ W2) + b2,
  h1 = relu(A_norm @ (x @ W1) + b1), A_norm = D^-1/2 (A+I) D^-1/2.
  mean is linear -> layer 2 collapses to a weighted sum over h1 rows:
    mean(h2) = (1/N) * (sum_n w_n * h1[n]) @ W2 + b2,
    w_n = dis_n * sum_{e: src_e = n} dis_{dst_e}
  and A_norm @ (x @ W1) = (A_norm @ x) @ W1, so message passing runs on the
  RAW (dis-scaled) x rows in fp8 and the W1 matmul happens per dst tile
  after aggregation:
    h1[n] = relu((dis_n * agg_n) @ W1 + b1), agg_n = sum_{e->n} dis_src*x_src

Device work per core (SPMD, same program, different data):
  dst nodes degree-sorted, assigned (core, slot, partition) as rank r ->
  core (r//P)%8, slot (r//P)//8, partition r%P. Per slot:
    1. one indirect-DMA gather lands each dst's edge-source xq rows (fp8,
       128B each) in its partition,
    2. DVE pairwise tree -> agg [128, 128] fp16,
    3. *dis_dst (tensor_scalar), transpose via PE identity matmul,
    4. h1 = aggT^T @ W1 + b1, relu,
    5. s += w_slot^T @ o1 into persistent [1,128] PSUM accumulator.
  host: sum 8 partials, /N, @W2, +b2.
"""

import sys

sys.path.insert(0, "/opt/trn_rl_repo")

from contextlib import ExitStack

import ml_dtypes
import numpy as np

import concourse.tile as tile
from concourse import bacc, bass, mybir
from concourse.bass import IndirectOffsetOnAxis
from concourse.bass_utils import run_bass_kernel_spmd

N = 50000
P = 128
NCORES = 8
TILES = 392            # ceil(N / P) rounded up to a multiple of NCORES
NPAD = TILES * P       # 50176
TPC = TILES // NCORES  # 49 dst tiles (slots) per core
D = 128                # feature dim (in & hidden)

F8 = mybir.dt.float8e4
F16 = mybir.dt.float16
F32 = mybir.dt.float32
I32 = mybir.dt.int32

CAST_GATHER = True  # fp8 DRAM -> fp16 SBUF cast during gather
N_PRE = 3           # tail (smallest) groups whose idx columns load first

_compiled = {}


def _groups(c_slots):
    """Pair adjacent slots (padded to a common capacity) + odd leftover."""
    gs = []
    u = 0
    while u + 1 < TPC:
        gs.append((u, u + 1))
        u += 2
    if u < TPC:
        gs.append((u,))
    widths = [
        len(g) * max(int(c_slots[t]) for t in g) for g in gs
    ]
    return gs, widths


def _build(c_slots, cast_gather=CAST_GATHER):
    """Build the Bass program. c_slots[t] = edge K-capacity of slot t."""
    groups, widths = _groups(c_slots)
    sumc = int(sum(widths))
    goffs = [0]
    for w_ in widths:
        goffs.append(goffs[-1] + w_)
    # groups processed smallest-first (reverse of degree-sorted order);
    # the last N_PRE groups' idx columns load first so gathers start early.
    split = int(goffs[-(N_PRE + 1)])
    wmax = int(max(widths))
    nc = bacc.Bacc(
        "TRN2", target_bir_lowering=False, debug=False, num_devices=NCORES
    )
    xq = nc.declare_dram_parameter("xq", [NPAD, D], F8, isOutput=False)
    w1 = nc.declare_dram_parameter("w1", [P, D], F16, isOutput=False)
    idxa = nc.declare_dram_parameter("idxa", [P, sumc - split], I32, isOutput=False)
    idxb = nc.declare_dram_parameter("idxb", [P, split], I32, isOutput=False)
    disv = nc.declare_dram_parameter("disv", [P, TPC], F32, isOutput=False)
    wv = nc.declare_dram_parameter("wv", [P, TPC], F16, isOutput=False)
    b1r = nc.declare_dram_parameter("b1r", [1, D], F16, isOutput=False)
    invd = nc.declare_dram_parameter("invd", [1, TPC * D], F16, isOutput=False)
    ident = nc.declare_dram_parameter("ident", [P, D], F16, isOutput=False)
    sout = nc.declare_dram_parameter("sout", [1, D], F32, isOutput=True)

    with tile.TileContext(nc) as tc, ExitStack() as ctx:
        const = ctx.enter_context(tc.tile_pool(name="const", bufs=1))
        gpool = ctx.enter_context(tc.tile_pool(name="gather", bufs=5))
        psT = ctx.enter_context(tc.tile_pool(name="psT", bufs=3, space="PSUM"))
        ps1 = ctx.enter_context(tc.tile_pool(name="ps1", bufs=3, space="PSUM"))
        sps = ctx.enter_context(tc.tile_pool(name="sps", bufs=1, space="PSUM"))
        epool = ctx.enter_context(tc.tile_pool(name="epi", bufs=6))
        opool = ctx.enter_context(tc.tile_pool(name="outp", bufs=1))

        # ---- resident constants; idxa first so slot-0 gather starts early ----
        idxa_sb = const.tile([P, sumc - split], I32)
        nc.sync.dma_start(out=idxa_sb[:], in_=idxa[:])
        w1_sb = const.tile([P, D], F16)
        nc.sync.dma_start(out=w1_sb[:], in_=w1[:])
        b1r_sb = const.tile([1, D], F16)
        nc.sync.dma_start(out=b1r_sb[:], in_=b1r[:])
        invd_sb = const.tile([1, TPC * D], F16)
        nc.sync.dma_start(out=invd_sb[:], in_=invd[:])
        ident_sb = const.tile([P, D], F16)
        nc.sync.dma_start(out=ident_sb[:], in_=ident[:])
        disv_sb = const.tile([P, TPC], F32)
        nc.sync.dma_start(out=disv_sb[:], in_=disv[:])
        wv_sb = const.tile([P, TPC], F16)
        nc.sync.dma_start(out=wv_sb[:], in_=wv[:])
        idxb_sb = const.tile([P, split], I32)
        nc.sync.dma_start(out=idxb_sb[:], in_=idxb[:])

        s_ps = sps.tile([1, D], F32)
        # 3 smallest groups first (fast fill), then descending width so the
        # run ends on small groups (short drain tail). Degree-sorted => width
        # is non-increasing with group index.
        ng = len(groups)
        order = list(range(ng - N_PRE, ng)) + list(range(ng - N_PRE))
        for k, gi in enumerate(order):
            g = groups[gi]
            width = widths[gi]
            cp = width // len(g)
            off = goffs[gi]
            src_sb, src_off = (
                (idxa_sb, off - split) if off >= split else (idxb_sb, off)
            )
            gath = gpool.tile([P, wmax * D], F16, tag="gather")
            nc.gpsimd.indirect_dma_start(
                out=gath[:, : width * D],
                out_offset=None,
                in_=xq[:],
                in_offset=IndirectOffsetOnAxis(
                    ap=src_sb[:, src_off : src_off + width], axis=0
                ),
            )
            # ---- segment sum: shared pairwise tree over the group ----
            if cp > 1:
                if len(g) == 2:
                    gv = gath[:, : width * D].rearrange(
                        "p (s e f) -> p s e f", s=2, e=cp
                    )
                    cur = cp
                    while cur > 1:
                        h = cur // 2
                        r = cur - h
                        nc.vector.tensor_tensor(
                            out=gv[:, :, 0:h, :],
                            in0=gv[:, :, 0:h, :],
                            in1=gv[:, :, r:cur, :],
                            op=mybir.AluOpType.add,
                        )
                        cur = r
                else:
                    cur = cp
                    while cur > 1:
                        h = cur // 2
                        r = cur - h
                        nc.vector.tensor_tensor(
                            out=gath[:, : h * D],
                            in0=gath[:, : h * D],
                            in1=gath[:, r * D : cur * D],
                            op=mybir.AluOpType.add,
                        )
                        cur = r

            # ---- epilogue per slot in the group ----
            for si, t in enumerate(g):
                # relu(dis*z + b1) = dis*relu(z + b1/dis): dis folds into wv
                # (host) and b1/dis is rank-1 -> the bias matmul; no scale op.
                agg_ap = gath[:, si * cp * D : si * cp * D + D]
                pT = psT.tile([P, D], F32, tag="pT")
                nc.tensor.matmul(
                    out=pT[:], lhsT=agg_ap, rhs=ident_sb[:],
                    start=True, stop=True,
                )
                aggT = epool.tile([P, D], F16, tag="aggT")
                nc.scalar.activation(
                    out=aggT[:], in_=pT[:],
                    func=mybir.ActivationFunctionType.Copy,
                )
                p1 = ps1.tile([P, D], F32, tag="p1")
                nc.tensor.matmul(
                    out=p1[:], lhsT=aggT[:], rhs=w1_sb[:],
                    start=True, stop=False,
                )
                # += (1/dis)^T @ b1 (rank-1 bias fold on PE)
                nc.tensor.matmul(
                    out=p1[:], lhsT=invd_sb[:, t * D : (t + 1) * D],
                    rhs=b1r_sb[:],
                    start=False, stop=True,
                )
                o1 = epool.tile([P, D], F16, tag="o1")
                nc.scalar.activation(
                    out=o1[:], in_=p1[:],
                    func=mybir.ActivationFunctionType.Relu,
                )
                nc.tensor.matmul(
                    out=s_ps[:],
                    lhsT=wv_sb[:, t : t + 1],
                    rhs=o1[:],
                    start=(k == 0 and si == 0),
                    stop=(k == len(order) - 1 and si == len(g) - 1),
                    skip_group_check=True,
                )

        s_sb = opool.tile([1, D], F32)
        nc.vector.tensor_copy(out=s_sb[:], in_=s_ps[:])
        nc.sync.dma_start(out=sout[:], in_=s_sb[:])

    nc.compile()
    return nc


def _prep(x, edge_index):
    """Host-side graph preprocessing -> per-core device input maps."""
    src = np.asarray(edge_index[0], dtype=np.int64)
    dst = np.asarray(edge_index[1], dtype=np.int64)
    loop = np.arange(N, dtype=np.int64)
    src_all = np.concatenate([src, loop])
    dst_all = np.concatenate([dst, loop])

    deg = np.bincount(dst_all, minlength=NPAD).astype(np.int64)
    dis = np.zeros(NPAD, dtype=np.float64)
    nz = deg > 0
    dis[nz] = 1.0 / np.sqrt(deg[nz])

    acc = np.zeros(NPAD, dtype=np.float64)
    np.add.at(acc, src_all, dis[dst_all])
    w = dis * acc  # layer-2 collapsed per-node weight

    # degree-sorted relabeling: rank r -> node perm[r];
    # tile rank rt = r // P -> core rt % 8, slot rt // 8, partition r % P.
    perm = np.argsort(-deg, kind="stable")
    rank = np.empty(NPAD, dtype=np.int64)
    rank[perm] = np.arange(NPAD)
    degs = deg[perm]
    c_slots = tuple(
        int(max(1, degs[(NCORES * t) * P])) for t in range(TPC)
    )
    groups, widths = _groups(c_slots)
    goffs = np.concatenate([[0], np.cumsum(widths)]).astype(np.int64)
    sumc = int(goffs[-1])
    col_base = np.zeros(TPC, dtype=np.int64)
    for gi, g in enumerate(groups):
        cp = widths[gi] // len(g)
        for si, t in enumerate(g):
            col_base[t] = goffs[gi] + si * cp

    # per-dst contiguous edge runs
    order = np.argsort(dst_all, kind="stable")
    src_s = src_all[order].astype(np.int32)
    dst_s = dst_all[order]
    starts = np.concatenate([[0], np.cumsum(np.bincount(dst_all, minlength=NPAD))])
    j = np.arange(dst_s.size, dtype=np.int64) - starts[dst_s]

    r = rank[dst_s]
    rt = r // P
    core = rt % NCORES
    slot = rt // NCORES
    p = r % P
    col = col_base[slot] + j

    idx_full = np.full((NCORES, P, sumc), N, dtype=np.int32)  # pad -> zero row
    idx_full[core, p, col] = src_s

    # per-core dis / w vectors in (partition, slot) layout
    node_of = perm.reshape(TILES, P)  # [tile rank, partition] -> node
    disv_full = np.empty((NCORES, P, TPC), dtype=np.float32)
    wv_full = np.empty((NCORES, P, TPC), dtype=np.float16)
    invd_full = np.zeros((NCORES, 1, TPC * D), dtype=np.float16)
    for k in range(NCORES):
        sel = node_of[k::NCORES, :]  # [TPC, P]
        dv = dis[sel]  # [TPC, P]
        disv_full[k] = dv.T.astype(np.float32)
        # dis folded into the s-weights; bias becomes b1/dis (rank-1 matmul)
        wv_full[k] = (w[sel] * dv).T.astype(np.float16)
        iv = np.where(dv > 0, 1.0 / np.maximum(dv, 1e-30), 0.0)
        invd_full[k][0] = iv.reshape(TPC * D).astype(np.float16)

    # xq[n] = dis_n * x_n, quantized to fp8 e4m3 (TRN max normal 240)
    xq = np.zeros((NPAD, D), dtype=ml_dtypes.float8_e4m3)
    xv = np.asarray(x, dtype=np.float64) * dis[:N, None]
    xq[:N] = np.clip(xv, -240, 240).astype(ml_dtypes.float8_e4m3)

    return c_slots, idx_full, disv_full, wv_full, invd_full, xq


def _make_in_maps(inputs):
    c_slots, idx_full, disv_full, wv_full, invd_full, xq = _prep(
        inputs["x"], inputs["edge_index"]
    )
    _, widths = _groups(c_slots)
    _split = int(sum(widths[:-N_PRE]))
    w1_d = np.asarray(inputs["W1"], dtype=np.float16)
    b1r = np.asarray(inputs["b1"], dtype=np.float16).reshape(1, D)

    ident = np.eye(P, D, dtype=np.float16)
    in_maps = []
    for k in range(NCORES):
        in_maps.append(
            {
                "xq": xq,
                "w1": w1_d,
                "idxa": np.ascontiguousarray(idx_full[k][:, _split:]),
                "idxb": np.ascontiguousarray(idx_full[k][:, :_split]),
                "disv": np.ascontiguousarray(disv_full[k]),
                "wv": np.ascontiguousarray(wv_full[k]),
                "b1r": b1r,
                "invd": np.ascontiguousarray(invd_full[k]),
                "ident": ident,
            }
        )
    return c_slots, in_maps


def _run(inputs, trace=False):
    c_slots, in_maps = _make_in_maps(inputs)
    if c_slots not in _compiled:
        _compiled[c_slots] = _build(c_slots)
    nc = _compiled[c_slots]

    res = run_bass_kernel_spmd(
        nc, in_maps, core_ids=list(range(NCORES)), trace=trace
    )
    s_total = np.zeros(D, dtype=np.float64)
    for r in res.results:
        s_total += r["sout"][0].astype(np.float64)

    out = (s_total / N) @ np.asarray(inputs["W2"], dtype=np.float64) + np.asarray(
        inputs["b2"], dtype=np.float64
    )
    return out[None, :].astype(np.float32), res.exec_time_ns


def kernel(x, edge_index, W1, b1, W2, b2):
    out, _ = _run(
        {
            "x": x,
            "edge_index": edge_index,
            "W1": W1,
            "b1": b1,
            "W2": W2,
            "b2": b2,
        }
    )
    return out
